# revision 1
# baseline (speedup 1.0000x reference)
"""Trainium2 Bass kernel: fused multi-head attention block (projections +
softmax attention + output projection + residual + LayerNorm).

Sharding: 8 cores = 2 batches x 4 query-chunks of 512. Each core computes
K/V for its whole batch (replicated within the 4-core batch group), Q only
for its 512-query chunk, full attention for that chunk over all 16 heads,
the output projection, residual add and LayerNorm. No collectives.

All cores run the same program; per-core inputs are pre-sliced on the host
with the key/value token order ROTATED so the core's query chunk occupies
rows 0..511 (attention is permutation-invariant over keys, and the key
padding mask is rotated identically).

Device-side layouts (per core):
  xt   [1024, 2048] bf16  x[b] transposed (feature-major), rotated
  xq   [512, 1024]  f32   query-chunk rows of x[b] (residual input)
  wq/wk/wv [1024, 1024] bf16  [c, h*64] (head-minor)
  wo   [1024, 1024] bf16  [(h*64+d), m]
  bias [16, 128]    f32   additive key mask bias per key tile/partition
  gamma/beta [1024] bf16
Output: y [512, 1024] f32.
"""

import contextlib

import numpy as np
import ml_dtypes

import concourse.bass as bass
import concourse.tile as tile
from concourse import mybir
from concourse import bass_utils

BF16 = ml_dtypes.bfloat16
N_CORES = 8
B, L, D, H, DH = 2, 2048, 1024, 16, 64
Q = L // 4          # queries per core
CT = D // 128       # contraction tiles over features
JT = L // 128       # key tiles
IT = Q // 128       # query tiles
LN_EPS = 1e-5

F32 = mybir.dt.float32
BF = mybir.dt.bfloat16


def _split_waits(nc, maxw=1):
    """This walrus build rejects instructions with more than one sync wait;
    split excess waits into preceding NOPs on the same engine."""
    ctr = 0
    for fn in nc.m.functions:
        for bb in fn.blocks:
            new_insts = []
            for inst in bb.instructions:
                si = inst.sync_info
                if si is not None and len(si.on_wait) > maxw:
                    waits = list(si.on_wait)
                    excess, keep = waits[:-maxw], waits[-maxw:]
                    for i in range(0, len(excess), maxw):
                        ctr += 1
                        new_insts.append(mybir.InstNoOp(
                            name=f"waitsplit_nop_{ctr}",
                            engine=inst.engine,
                            sync_info=mybir.SyncInfo(
                                on_wait=excess[i:i + maxw], on_update=[]),
                            text_hint="waitsplit",
                        ))
                    si.on_wait = keep
                new_insts.append(inst)
            bb.instructions = new_insts
    return ctr


def _bcast_parts(ap, parts):
    """Broadcast-read view of a [1, N] AP as [1, parts, N] via a stride-0
    free dim (SBUF APs may not have a stride-0 partition dim)."""
    return bass.AP(tensor=ap.tensor, offset=ap.offset,
                   ap=[list(ap.ap)[0], [0, parts]] + list(ap.ap)[1:])


def _emit(nc, tc, hh, masked):
    Exp = mybir.ActivationFunctionType.Exp
    Sqrt = mybir.ActivationFunctionType.Sqrt

    xt_ap = hh["xt"].ap().rearrange("(t p) l -> p t l", p=128)      # [128,8,2048]
    wq_ap = hh["wq"].ap().rearrange("(t p) d -> p t d", p=128)
    wk_ap = hh["wk"].ap().rearrange("(t p) d -> p t d", p=128)
    wv_ap = hh["wv"].ap().rearrange("(t p) d -> p t d", p=128)
    wo_ap = hh["wo"].ap().rearrange("(t p) d -> p t d", p=128)
    bias_ap = hh["bias"].ap().rearrange("a b -> b a")               # [128,16]
    xq_ap = hh["xq"].ap()
    y_ap = hh["y"].ap()

    def bcast_dram(h1d, parts=128):
        a = h1d.ap()
        return bass.AP(tensor=a.tensor, offset=a.offset,
                       ap=[[0, parts]] + list(a.ap))

    with contextlib.ExitStack() as ctx:
        const = ctx.enter_context(tc.tile_pool(name="const", bufs=1))
        wpool = ctx.enter_context(tc.tile_pool(name="wpool", bufs=2))
        xtp = ctx.enter_context(tc.tile_pool(name="xtp", bufs=2))
        expp = ctx.enter_context(tc.tile_pool(name="expp", bufs=2))
        ktp = ctx.enter_context(tc.tile_pool(name="ktp", bufs=3))
        vp = ctx.enter_context(tc.tile_pool(name="vp", bufs=1))
        qtp = ctx.enter_context(tc.tile_pool(name="qtp", bufs=1))
        ptp = ctx.enter_context(tc.tile_pool(name="ptp", bufs=1))
        npool = ctx.enter_context(tc.tile_pool(name="npool", bufs=3))
        xqp = ctx.enter_context(tc.tile_pool(name="xqp", bufs=1))
        lnp = ctx.enter_context(tc.tile_pool(name="lnp", bufs=3))
        statp = ctx.enter_context(tc.tile_pool(name="statp", bufs=4))
        psS = ctx.enter_context(tc.tile_pool(name="psS", bufs=2, space="PSUM"))
        psP = ctx.enter_context(tc.tile_pool(name="psP", bufs=2, space="PSUM"))
        psD = ctx.enter_context(tc.tile_pool(name="psD", bufs=2, space="PSUM"))

        # ---- constants / small loads ----
        eps_sb = const.tile([128, 1], F32)
        nc.vector.memset(eps_sb[:], LN_EPS)
        ones64 = const.tile([1, 64], F32)
        nc.vector.memset(ones64[:], 1.0)
        bias_sb = const.tile([128, 16], F32)
        nc.gpsimd.dma_start(out=bias_sb[:], in_=bias_ap)
        gamma_sb = const.tile([128, 1024], BF)
        beta_sb = const.tile([128, 1024], BF)

        # ---- big SBUF tensors ----
        v_all = vp.tile([128, JT, H, DH + 1], BF)  # V by key tile, +ones col
        qt_all = qtp.tile([128, 8, Q], BF)         # Q^T for the chunk
        probt = ptp.tile([128, 8, Q], BF)          # normalized P^T stacked

        nc.vector.memset(v_all[:, :, :, DH:DH + 1], 1.0)

        # weights streamed through wpool: wv, wq first; wk, wo reuse slots
        # (weights go on the ACT HWDGE queue, xt on the SP queue, so the
        # two initial load streams run in parallel)
        xt0 = xtp.tile([128, 4, 2048], BF, tag="xt")
        nc.sync.dma_start(out=xt0[:], in_=xt_ap[:, 0:4, :])
        wv_sb = wpool.tile([128, 8, 1024], BF, tag="w")
        nc.scalar.dma_start(out=wv_sb[:], in_=wv_ap)
        xt1 = xtp.tile([128, 4, 2048], BF, tag="xt")
        nc.scalar.dma_start(out=xt1[:], in_=xt_ap[:, 4:8, :])
        wq_sb = wpool.tile([128, 8, 1024], BF, tag="w")
        nc.sync.dma_start(out=wq_sb[:], in_=wq_ap)

        def xt_sl(ct, lo, size):
            t = xt0 if ct < 4 else xt1
            return t[:, ct % 4, lo:lo + size]

        # ---- V projection: [token 128][h*64] ----
        for lt in range(JT):
            ps = psS.tile([128, 2, 512], F32, tag="ss")
            for nt in range(2):
                for ct in range(CT):
                    nc.tensor.matmul(
                        ps[:, nt, :], xt_sl(ct, lt * 128, 128),
                        wv_sb[:, ct, nt * 512:(nt + 1) * 512],
                        start=(ct == 0), stop=(ct == CT - 1))
            nc.vector.tensor_copy(
                v_all[:, lt, :, 0:DH],
                ps.rearrange("p n (h d) -> p (n h) d", h=8))

        wk_sb = wpool.tile([128, 8, 1024], BF, tag="w")
        nc.scalar.dma_start(out=wk_sb[:], in_=wk_ap)

        # ---- Q^T projection: [d' 128][i 512] ----
        for dtp in range(4):
            ps = psS.tile([128, 2, 512], F32, tag="ss")
            for half in range(2):
                dt = 2 * dtp + half
                for ct in range(CT):
                    nc.tensor.matmul(
                        ps[:, half, :], wq_sb[:, ct, dt * 128:(dt + 1) * 128],
                        xt_sl(ct, 0, Q),
                        start=(ct == 0), stop=(ct == CT - 1))
            nc.vector.tensor_copy(qt_all[:, 2 * dtp:2 * dtp + 2, :], ps[:])

        wo_sb = wpool.tile([128, 8, 1024], BF, tag="w")
        nc.scalar.dma_start(out=wo_sb[:], in_=wo_ap)

        # ---- K^T projection (per d'-tile) interleaved with attention ----
        for dt in range(8):
            kt_t = ktp.tile([128, 2048], BF, tag="kt")
            for jp in range(2):
                ps = psS.tile([128, 2, 512], F32, tag="ss")
                for j4 in range(2):
                    for ct in range(CT):
                        nc.tensor.matmul(
                            ps[:, j4, :],
                            wk_sb[:, ct, dt * 128:(dt + 1) * 128],
                            xt_sl(ct, jp * 1024 + j4 * 512, 512),
                            start=(ct == 0), stop=(ct == CT - 1))
                # kt evictions go on ACT: DVE's queue carries the 3.3us
                # reciprocals, which would stall the next d'-tile's logits.
                nc.scalar.copy(
                    kt_t[:, jp * 1024:(jp + 1) * 1024], ps[:])

            # Both heads of this d'-tile run as concurrent row-group
            # matmuls (K=64 in rows 0-63 and 64-127 of the PE array),
            # sharing one batched exp per key tile.
            expt_halves = []
            for half in range(2):
                expt = expp.tile([128, JT // 2, 2, 512], BF, tag="e")
                expt_halves.append(expt)
                for jj in range(JT // 2):
                    jt = half * (JT // 2) + jj
                    ps = psS.tile([128, 2, 512], F32, tag="ss")
                    for hb in range(2):
                        nc.tensor.matmul(
                            ps[:, hb, :],
                            kt_t[hb * 64:hb * 64 + 64,
                                 jt * 128:(jt + 1) * 128],
                            qt_all[hb * 64:hb * 64 + 64, dt, :],
                            start=True, stop=True)
                    if masked:
                        for hb in range(2):
                            nc.scalar.activation(
                                expt[:, jj, hb, :], ps[:, hb, :], Exp,
                                bias=bias_sb[:, jt:jt + 1], scale=1.0 / 8.0)
                    else:
                        nc.scalar.activation(
                            expt[:, jj, :, :], ps[:], Exp,
                            bias=0.0, scale=1.0 / 8.0)
            # PV: interleave the two heads' accumulation chains so the PE
            # alternates PSUM banks (hides the same-bank drain latency).
            pv_ps = [psP.tile([DH + 1, 512], F32, tag="pp", name=f"pv{hb}")
                     for hb in range(2)]
            for jt in range(JT):
                for hb in range(2):
                    nc.tensor.matmul(
                        pv_ps[hb][:], v_all[:, jt, 2 * dt + hb, 0:DH + 1],
                        expt_halves[jt // (JT // 2)][:, jt % (JT // 2), hb, :],
                        start=(jt == 0), stop=(jt == JT - 1))
            for hb in range(2):
                poff = hb * 64
                ps_p = pv_ps[hb]
                den = npool.tile([1, 512], F32, tag="n")
                nc.scalar.copy(den[:], ps_p[DH:DH + 1, :])
                ps_d = psD.tile([64, 512], F32, tag="dd")
                nc.tensor.matmul(ps_d[:], ones64[:], den[:],
                                 start=True, stop=True)
                rdiv = npool.tile([64, 512], F32, tag="n")
                nc.vector.reciprocal(rdiv[:], ps_d[:])
                nc.vector.tensor_mul(
                    probt[poff:poff + 64, dt, :], ps_p[0:DH, :], rdiv[:])

        # ---- output projection + residual + LayerNorm ----
        nc.gpsimd.dma_start(out=gamma_sb[:], in_=bcast_dram(hh["gamma"]))
        nc.gpsimd.dma_start(out=beta_sb[:], in_=bcast_dram(hh["beta"]))
        for it in range(IT):
            xq_t = xqp.tile([128, 1024], F32, tag="xq")
            nc.sync.dma_start(out=xq_t[:],
                              in_=xq_ap[it * 128:(it + 1) * 128, :])
            ps_r = psS.tile([128, 2, 512], F32, tag="ss")
            for mh in range(2):
                for kt in range(8):
                    nc.tensor.matmul(
                        ps_r[:, mh, :],
                        probt[:, kt, it * 128:(it + 1) * 128],
                        wo_sb[:, kt, mh * 512:(mh + 1) * 512],
                        start=(kt == 0), stop=(kt == 7))
            h_sb = lnp.tile([128, 1024], F32, tag="ln")
            nc.vector.tensor_add(h_sb[:], ps_r.rearrange("p a b -> p (a b)"),
                                 xq_t[:])
            stats = statp.tile([128, 2, 6], F32)
            nc.vector.bn_stats(stats[:, 0, :], h_sb[:, 0:512])
            nc.vector.bn_stats(stats[:, 1, :], h_sb[:, 512:1024])
            mv = statp.tile([128, 2], F32)
            nc.vector.bn_aggr(mv[:], stats[:])
            std = statp.tile([128, 1], F32)
            nc.scalar.activation(std[:], mv[:, 1:2], Sqrt,
                                 bias=eps_sb[:], scale=1.0)
            rstd = statp.tile([128, 1], F32)
            nc.vector.reciprocal(rstd[:], std[:])
            t1 = lnp.tile([128, 1024], F32, tag="ln")
            nc.vector.tensor_scalar(
                t1[:], h_sb[:], mv[:, 0:1], rstd[:],
                op0=mybir.AluOpType.subtract, op1=mybir.AluOpType.mult)
            t2 = lnp.tile([128, 1024], F32, tag="ln")
            nc.vector.tensor_mul(t2[:], t1[:], gamma_sb[:])
            out_t = lnp.tile([128, 1024], F32, tag="ln")
            nc.vector.tensor_add(out_t[:], t2[:], beta_sb[:])
            nc.sync.dma_start(y_ap[it * 128:(it + 1) * 128, :], out_t[:])


def build_module(split=True, masked=False):
    nc = bass.Bass("TRN2", target_bir_lowering=False, debug=False,
                   num_devices=N_CORES)
    hh = {
        "xt": nc.dram_tensor("xt", [D, L], BF, kind="ExternalInput"),
        "xq": nc.dram_tensor("xq", [Q, D], F32, kind="ExternalInput"),
        "wq": nc.dram_tensor("wq", [D, D], BF, kind="ExternalInput"),
        "wk": nc.dram_tensor("wk", [D, D], BF, kind="ExternalInput"),
        "wv": nc.dram_tensor("wv", [D, D], BF, kind="ExternalInput"),
        "wo": nc.dram_tensor("wo", [D, D], BF, kind="ExternalInput"),
        "bias": nc.dram_tensor("bias", [16, 128], F32, kind="ExternalInput"),
        "gamma": nc.dram_tensor("gamma", [D], BF, kind="ExternalInput"),
        "beta": nc.dram_tensor("beta", [D], BF, kind="ExternalInput"),
        "y": nc.dram_tensor("y", [Q, D], F32, kind="ExternalOutput"),
    }
    with tile.TileContext(nc) as tc:
        _emit(nc, tc, hh, masked)
    if split:
        _split_waits(nc, 1)
    return nc


_CACHE = {}


def get_module(masked=False):
    key = ("nc", masked)
    if key not in _CACHE:
        _CACHE[key] = build_module(masked=masked)
    return _CACHE[key]


def prep_inputs(x, mask, w_q, w_k, w_v, w_o, ln_gamma, ln_beta):
    x = np.asarray(x, dtype=np.float32)
    mask = np.asarray(mask)
    shared = {
        "wq": np.ascontiguousarray(
            np.asarray(w_q, np.float32).transpose(1, 0, 2).reshape(D, D)
        ).astype(BF16),
        "wk": np.ascontiguousarray(
            np.asarray(w_k, np.float32).transpose(1, 0, 2).reshape(D, D)
        ).astype(BF16),
        "wv": np.ascontiguousarray(
            np.asarray(w_v, np.float32).transpose(1, 0, 2).reshape(D, D)
        ).astype(BF16),
        "wo": np.asarray(w_o, np.float32).reshape(D, D).astype(BF16),
        "gamma": np.asarray(ln_gamma, np.float32).astype(BF16),
        "beta": np.asarray(ln_beta, np.float32).astype(BF16),
    }
    in_maps = []
    for c in range(N_CORES):
        b, q0 = c // 4, (c % 4) * Q
        perm = np.r_[q0:L, 0:q0]
        xb = x[b][perm]                       # rotated: q-chunk first
        m = {
            "xt": np.ascontiguousarray(xb.T).astype(BF16),
            "xq": np.ascontiguousarray(x[b, q0:q0 + Q, :]),
            "bias": np.where(mask[b][perm], 0.0, -1e9).astype(
                np.float32).reshape(16, 128),
        }
        m.update(shared)
        in_maps.append(m)
    masked = not bool(mask.all())
    return in_maps, masked


def assemble(results):
    out = np.empty((B, L, D), dtype=np.float32)
    for c in range(N_CORES):
        b, q0 = c // 4, (c % 4) * Q
        out[b, q0:q0 + Q, :] = results[c]["y"]
    return out


def run(in_maps, masked=False, **kwargs):
    nc = get_module(masked)
    return bass_utils.run_bass_kernel_spmd(
        nc, in_maps, core_ids=list(range(N_CORES)), **kwargs)


def kernel(x, mask, w_q, w_k, w_v, w_o, ln_gamma, ln_beta):
    in_maps, masked = prep_inputs(x, mask, w_q, w_k, w_v, w_o,
                                  ln_gamma, ln_beta)
    res = run(in_maps, masked)
    return assemble(res.results)



# revision 2
# speedup vs baseline: 1.1609x; 1.1609x over previous
"""Trainium2 Bass kernel v5: fused MHA block, sequence-parallel queries with
replicated K projection and a single early V AllGather.

Sharding: 8 cores = 2 batches x 4 query-chunks of 512. Each core:
  - computes Q^T for its own 512 queries,
  - computes V for its own 512 tokens and AllGathers V across the 4-core
    batch group (1MB bf16, triggered at ~t=25us, needed by PV at ~t=90us),
  - computes K^T for ALL 2048 batch tokens locally (replicated: +41us PE,
    but keeps the QK->exp pipeline free of any collective dependency),
  - runs 16-head attention for its queries, output projection, residual,
    LayerNorm.

The softmax normalize is deferred: PV accumulates with a ones-column (so
PSUM row 64 is the denominator), partials are evicted to SBUF, and one
[128,512] reciprocal per head-pair (both heads broadcast into one PSUM
bank) replaces the baseline's 2x [64,512] reciprocals.

Per-core inputs:
  xtq  [1024, 512]  bf16  x[b,chunk].T (Q/V projections)
  xtk  [1024, 2048] bf16  x[b].T full batch (K projection)
  xq   [512, 1024]  f32   chunk rows of x[b] (residual input)
  wq/wk/wv [1024, 1024] bf16  [c, h*64] head-minor
  wo   [1024, 1024] bf16
  bias [16, 128]    f32   additive key bias (masked path), natural order
  gamma/beta [1024] bf16
Output: y [512, 1024] f32.
"""

import contextlib

import numpy as np
import ml_dtypes

import concourse.bass as bass
import concourse.tile as tile
from concourse import mybir
from concourse import bass_utils

BF16 = ml_dtypes.bfloat16
N_CORES = 8
B, L, D, H, DH = 2, 2048, 1024, 16, 64
C = 512             # queries per core (chunk)
CT = D // 128       # contraction tiles over features
JT = L // 128       # key tiles
IT = C // 128       # query tiles per core
LN_EPS = 1e-5

F32 = mybir.dt.float32
BF = mybir.dt.bfloat16

GROUPS = [[0, 1, 2, 3], [4, 5, 6, 7]]


def _split_waits(nc, maxw=1):
    """This walrus build rejects instructions with more than one sync wait;
    split excess waits into preceding NOPs on the same engine."""
    ctr = 0
    for fn in nc.m.functions:
        for bb in fn.blocks:
            new_insts = []
            for inst in bb.instructions:
                si = inst.sync_info
                if si is not None and len(si.on_wait) > maxw:
                    waits = list(si.on_wait)
                    excess, keep = waits[:-maxw], waits[-maxw:]
                    for i in range(0, len(excess), maxw):
                        ctr += 1
                        new_insts.append(mybir.InstNoOp(
                            name=f"waitsplit_nop_{ctr}",
                            engine=inst.engine,
                            sync_info=mybir.SyncInfo(
                                on_wait=excess[i:i + maxw], on_update=[]),
                            text_hint="waitsplit",
                        ))
                    si.on_wait = keep
                new_insts.append(inst)
            bb.instructions = new_insts
    return ctr


def _emit(nc, tc, hh, masked):
    Exp = mybir.ActivationFunctionType.Exp
    Sqrt = mybir.ActivationFunctionType.Sqrt

    xtq_ap = hh["xtq"].ap().rearrange("(t p) k -> p t k", p=128)  # [128,8,512]
    xtk_ap = hh["xtk"].ap().rearrange("(t p) k -> p t k", p=128)  # [128,8,2048]
    wq_ap = hh["wq"].ap().rearrange("(t p) d -> p t d", p=128)
    wk_ap = hh["wk"].ap().rearrange("(t p) d -> p t d", p=128)
    wv_ap = hh["wv"].ap().rearrange("(t p) d -> p t d", p=128)
    wo_ap = hh["wo"].ap().rearrange("(t p) d -> p t d", p=128)
    bias_ap = hh["bias"].ap().rearrange("a b -> b a")             # [128,16]
    xq_ap = hh["xq"].ap()
    y_ap = hh["y"].ap()
    vb_ap = hh["vb"].ap().rearrange("(t p) d -> p t d", p=128)    # [128,4,1024]
    # gathered V: [2048, 1024] -> per key tile [128, h, 64]
    vg_ap = hh["vg"].ap().rearrange("(j p) (h d) -> p j h d", p=128, d=DH)

    def bcast_dram(h1d, parts=128):
        a = h1d.ap()
        return bass.AP(tensor=a.tensor, offset=a.offset,
                       ap=[[0, parts]] + list(a.ap))

    with contextlib.ExitStack() as ctx:
        const = ctx.enter_context(tc.tile_pool(name="const", bufs=1))
        wpool = ctx.enter_context(tc.tile_pool(name="wpool", bufs=3))
        xtp = ctx.enter_context(tc.tile_pool(name="xtp", bufs=1))
        xkp = ctx.enter_context(tc.tile_pool(name="xkp", bufs=3))
        ktp = ctx.enter_context(tc.tile_pool(name="ktp", bufs=1))
        vp = ctx.enter_context(tc.tile_pool(name="vp", bufs=1))
        exp_ = ctx.enter_context(tc.tile_pool(name="exp", bufs=4))
        qtp = ctx.enter_context(tc.tile_pool(name="qtp", bufs=1))
        ptp = ctx.enter_context(tc.tile_pool(name="ptp", bufs=1))
        pv2p = ctx.enter_context(tc.tile_pool(name="pv2p", bufs=2))
        npool = ctx.enter_context(tc.tile_pool(name="npool", bufs=1))
        xqp = ctx.enter_context(tc.tile_pool(name="xqp", bufs=2))
        lnp = ctx.enter_context(tc.tile_pool(name="lnp", bufs=2))
        statp = ctx.enter_context(tc.tile_pool(name="statp", bufs=4))
        psS = ctx.enter_context(tc.tile_pool(name="psS", bufs=2, space="PSUM"))
        psP = ctx.enter_context(tc.tile_pool(name="psP", bufs=2, space="PSUM"))
        psO = ctx.enter_context(tc.tile_pool(name="psO", bufs=1, space="PSUM"))

        # ---- constants / small loads ----
        eps_sb = const.tile([128, 1], F32)
        nc.vector.memset(eps_sb[:], LN_EPS)
        ones64 = const.tile([1, 64], BF)
        nc.vector.memset(ones64[:], 1.0)
        bias_sb = const.tile([128, 16], F32)
        nc.gpsimd.dma_start(out=bias_sb[:], in_=bias_ap)
        gamma_sb = const.tile([128, 1024], BF)
        beta_sb = const.tile([128, 1024], BF)

        # ---- persistent SBUF tensors ----
        xtq_sb = xtp.tile([128, 8, 512], BF)
        qt_all = qtp.tile([128, 8, 512], BF)
        probt = ptp.tile([128, 8, 512], BF)
        kt_all = ktp.tile([128, 8, 2048], BF)      # K^T all batch tokens
        v_all = vp.tile([128, JT, H, DH + 1], BF)  # gathered V + ones col
        v_loc = xtp.tile([128, 4, 1024], BF)       # own V (AG source)

        nc.vector.memset(v_all[:, :, :, DH:DH + 1], 1.0)

        # initial loads: xtq + wv first (V projection gates the collective)
        nc.sync.dma_start(out=xtq_sb[:], in_=xtq_ap)

        def whalf(w_ap, h, queue):
            t = wpool.tile([128, 8, 512], BF, tag="w", name=f"w{h}")
            queue(out=t[:], in_=w_ap[:, :, h * 512:(h + 1) * 512])
            return t

        wv_h = [whalf(wv_ap, h, nc.scalar.dma_start) for h in range(2)]
        wq_h = [whalf(wq_ap, h, nc.scalar.dma_start) for h in range(2)]
        wk_h = [whalf(wk_ap, h, nc.scalar.dma_start) for h in range(2)]

        # ---- V projection (own chunk) + AllGather ----
        for tt in range(4):
            ps = psS.tile([128, 2, 512], F32, tag="ss")
            for nh in range(2):
                for ct in range(CT):
                    nc.tensor.matmul(
                        ps[:, nh, :], xtq_sb[:, ct, tt * 128:(tt + 1) * 128],
                        wv_h[nh][:, ct, :],
                        start=(ct == 0), stop=(ct == CT - 1))
            nc.vector.tensor_copy(
                v_loc[:, tt, :], ps.rearrange("p a b -> p (a b)"))
            nc.gpsimd.dma_start(out=vb_ap[:, tt, :], in_=v_loc[:, tt, :])
        nc.gpsimd.collective_compute(
            "AllGather", mybir.AluOpType.bypass, replica_groups=GROUPS,
            ins=[hh["vb"].ap()], outs=[hh["vg"].ap()])
        # ---- Q^T projection ----
        for p2 in range(4):
            ps = psS.tile([128, 2, 512], F32, tag="ss")
            for half in range(2):
                dt = 2 * p2 + half
                for ct in range(CT):
                    nc.tensor.matmul(
                        ps[:, half, :],
                        wq_h[dt // 4][:, ct, (dt % 4) * 128:(dt % 4 + 1) * 128],
                        xtq_sb[:, ct, :],
                        start=(ct == 0), stop=(ct == CT - 1))
            nc.vector.tensor_copy(qt_all[:, 2 * p2:2 * p2 + 2, :], ps[:])

        # ---- K^T projection, replicated over the full batch ----
        # chunk-at-a-time so QK on chunk g can start as soon as it's done
        def kproj_chunk(g):
            xtk_sb = xkp.tile([128, 8, 512], BF, tag="xk", name=f"xtk{g}")
            nc.sync.dma_start(out=xtk_sb[:],
                              in_=xtk_ap[:, :, g * 512:(g + 1) * 512])
            for p2 in range(4):
                ps = psS.tile([128, 2, 512], F32, tag="ss")
                for half in range(2):
                    dt = 2 * p2 + half
                    for ct in range(CT):
                        nc.tensor.matmul(
                            ps[:, half, :],
                            wk_h[dt // 4][:, ct,
                                          (dt % 4) * 128:(dt % 4 + 1) * 128],
                            xtk_sb[:, ct, :],
                            start=(ct == 0), stop=(ct == CT - 1))
                nc.vector.tensor_copy(
                    kt_all[:, 2 * p2:2 * p2 + 2, g * 512:(g + 1) * 512],
                    ps[:])

        # gathered V -> SBUF, strided into the [h, 65] ones-column layout
        # (emitted after the K projection so these collective-gated loads
        # sit behind the xtk chunk loads in the SP queue)
        for jt in range(JT):
            nc.gpsimd.dma_start(out=v_all[:, jt, :, 0:DH],
                              in_=vg_ap[:, jt, :, :])

        for g in range(4):
            kproj_chunk(g)

        # ---- attention per head-pair dt ----
        def attention_dt(dt, interleave_kproj=False):
            expts = [exp_.tile([128, 4, 2, 512], BF, name=f"e{dt}_{i}",
                               tag="e") for i in range(4)]
            pv_ps = [psP.tile([DH + 1, 512], F32, tag="pp",
                              name=f"pv{dt}_{hb}") for hb in range(2)]

            def qk_jt(jt):
                expt = expts[jt // 4]
                ps = psS.tile([128, 2, 512], F32, tag="ss")
                for hb in range(2):
                    nc.tensor.matmul(
                        ps[:, hb, :],
                        kt_all[hb * 64:hb * 64 + 64, dt,
                               jt * 128:(jt + 1) * 128],
                        qt_all[hb * 64:hb * 64 + 64, dt, :],
                        start=True, stop=True)
                if masked:
                    nc.scalar.activation(
                        expt[:, jt % 4, :, :], ps[:], Exp,
                        bias=bias_sb[:, jt:jt + 1], scale=1.0 / 8.0)
                else:
                    nc.scalar.activation(
                        expt[:, jt % 4, :, :], ps[:], Exp,
                        bias=0.0, scale=1.0 / 8.0)

            def pv_jt(jt):
                for hb in range(2):
                    nc.tensor.matmul(
                        pv_ps[hb][:],
                        v_all[:, jt, 2 * dt + hb, 0:DH + 1],
                        expts[jt // 4][:, jt % 4, hb, :],
                        start=(jt == 0), stop=(jt == JT - 1))

            # PV lags QK by 2 tiles: the PE-queue waits land on already-
            # fired exp semaphores, so LDWEIGHTS keeps flowing.
            for jt in range(JT):
                qk_jt(jt)
                if jt >= 2:
                    pv_jt(jt - 2)
            pv_jt(JT - 2)
            pv_jt(JT - 1)
            # deferred normalize: evict partials, one [128,512] reciprocal
            pv2 = pv2p.tile([DH, 2, 512], F32, tag="pv2", name=f"pv2_{dt}")
            den = npool.tile([1, 2, 512], BF, tag="n1", name=f"den{dt}")
            for hb in range(2):
                nc.vector.tensor_copy(pv2[:, hb, :], pv_ps[hb][0:DH, :])
                nc.vector.tensor_copy(den[:, hb, :], pv_ps[hb][DH:DH + 1, :])
            ps_d = psO.tile([128, 2, 512], F32, tag="oo",
                            name=f"psd{dt}")[:, 0, :]
            nc.tensor.matmul(ps_d[0:64, :], ones64[:], den[:, 0, :],
                             start=True, stop=True)
            nc.tensor.matmul(ps_d[64:128, :], ones64[:], den[:, 1, :],
                             start=True, stop=True)
            nc.vector.reciprocal(ps_d[:], ps_d[:])
            for hb in range(2):
                nc.vector.tensor_mul(
                    probt[hb * 64:hb * 64 + 64, dt, :], pv2[:, hb, :],
                    ps_d[hb * 64:hb * 64 + 64, :])

        for dt in range(8):
            attention_dt(dt)

        # ---- output projection + residual + LayerNorm ----
        nc.gpsimd.dma_start(out=gamma_sb[:], in_=bcast_dram(hh["gamma"]))
        nc.gpsimd.dma_start(out=beta_sb[:], in_=bcast_dram(hh["beta"]))
        wo_h = [whalf(wo_ap, h, nc.scalar.dma_start) for h in range(2)]
        for it in range(IT):
            xq_t = xqp.tile([128, 1024], F32, tag="xq")
            nc.scalar.dma_start(out=xq_t[:],
                                in_=xq_ap[it * 128:(it + 1) * 128, :])
            ps_r = psO.tile([128, 2, 512], F32, tag="oo")
            for mh in range(2):
                for kt in range(8):
                    nc.tensor.matmul(
                        ps_r[:, mh, :],
                        probt[:, kt, it * 128:(it + 1) * 128],
                        wo_h[mh][:, kt, :],
                        start=(kt == 0), stop=(kt == 7))
            h_sb = lnp.tile([128, 1024], F32, tag="ln")
            nc.vector.tensor_add(h_sb[:], ps_r.rearrange("p a b -> p (a b)"),
                                 xq_t[:])
            stats = statp.tile([128, 2, 6], F32)
            nc.vector.bn_stats(stats[:, 0, :], h_sb[:, 0:512])
            nc.vector.bn_stats(stats[:, 1, :], h_sb[:, 512:1024])
            mv = statp.tile([128, 2], F32)
            nc.vector.bn_aggr(mv[:], stats[:])
            std = statp.tile([128, 1], F32)
            nc.scalar.activation(std[:], mv[:, 1:2], Sqrt,
                                 bias=eps_sb[:], scale=1.0)
            rstd = statp.tile([128, 1], F32)
            nc.vector.reciprocal(rstd[:], std[:])
            t1 = lnp.tile([128, 1024], F32, tag="ln")
            nc.vector.tensor_scalar(
                t1[:], h_sb[:], mv[:, 0:1], rstd[:],
                op0=mybir.AluOpType.subtract, op1=mybir.AluOpType.mult)
            t2 = lnp.tile([128, 1024], F32, tag="ln")
            nc.vector.tensor_mul(t2[:], t1[:], gamma_sb[:])
            out_t = lnp.tile([128, 1024], F32, tag="ln")
            nc.vector.tensor_add(out_t[:], t2[:], beta_sb[:])
            nc.scalar.dma_start(y_ap[it * 128:(it + 1) * 128, :], out_t[:])


def build_module(split=True, masked=False):
    nc = bass.Bass("TRN2", target_bir_lowering=False, debug=False,
                   num_devices=N_CORES)
    hh = {
        "xtq": nc.dram_tensor("xtq", [D, C], BF, kind="ExternalInput"),
        "xtk": nc.dram_tensor("xtk", [D, L], BF, kind="ExternalInput"),
        "xq": nc.dram_tensor("xq", [C, D], F32, kind="ExternalInput"),
        "wq": nc.dram_tensor("wq", [D, D], BF, kind="ExternalInput"),
        "wk": nc.dram_tensor("wk", [D, D], BF, kind="ExternalInput"),
        "wv": nc.dram_tensor("wv", [D, D], BF, kind="ExternalInput"),
        "wo": nc.dram_tensor("wo", [D, D], BF, kind="ExternalInput"),
        "bias": nc.dram_tensor("bias", [16, 128], F32, kind="ExternalInput"),
        "gamma": nc.dram_tensor("gamma", [D], BF, kind="ExternalInput"),
        "beta": nc.dram_tensor("beta", [D], BF, kind="ExternalInput"),
        "y": nc.dram_tensor("y", [C, D], F32, kind="ExternalOutput"),
        "vb": nc.dram_tensor("vb", [C, D], BF, kind="Internal"),
        "vg": nc.dram_tensor("vg", [L, D], BF, kind="Internal"),
    }
    with tile.TileContext(nc) as tc:
        _emit(nc, tc, hh, masked)
    if split:
        _split_waits(nc, 1)
    return nc


_CACHE = {}


def get_module(masked=False):
    key = ("nc", masked)
    if key not in _CACHE:
        _CACHE[key] = build_module(masked=masked)
    return _CACHE[key]


def prep_inputs(x, mask, w_q, w_k, w_v, w_o, ln_gamma, ln_beta):
    x = np.asarray(x, dtype=np.float32)
    mask = np.asarray(mask)
    shared = {
        "wq": np.ascontiguousarray(
            np.asarray(w_q, np.float32).transpose(1, 0, 2).reshape(D, D)
        ).astype(BF16),
        "wk": np.ascontiguousarray(
            np.asarray(w_k, np.float32).transpose(1, 0, 2).reshape(D, D)
        ).astype(BF16),
        "wv": np.ascontiguousarray(
            np.asarray(w_v, np.float32).transpose(1, 0, 2).reshape(D, D)
        ).astype(BF16),
        "wo": np.asarray(w_o, np.float32).reshape(D, D).astype(BF16),
        "gamma": np.asarray(ln_gamma, np.float32).astype(BF16),
        "beta": np.asarray(ln_beta, np.float32).astype(BF16),
    }
    xtk = {b: np.ascontiguousarray(x[b].T).astype(BF16) for b in range(B)}
    in_maps = []
    for c in range(N_CORES):
        b, i = c // 4, c % 4
        q0 = i * C
        m = {
            "xtq": np.ascontiguousarray(x[b, q0:q0 + C, :].T).astype(BF16),
            "xtk": xtk[b],
            "xq": np.ascontiguousarray(x[b, q0:q0 + C, :]),
            "bias": np.where(mask[b], 0.0, -1e9).astype(
                np.float32).reshape(16, 128),
        }
        m.update(shared)
        in_maps.append(m)
    masked = not bool(mask.all())
    return in_maps, masked


def assemble(results):
    out = np.empty((B, L, D), dtype=np.float32)
    for c in range(N_CORES):
        b, q0 = c // 4, (c % 4) * C
        out[b, q0:q0 + C, :] = results[c]["y"]
    return out


def run(in_maps, masked=False, **kwargs):
    nc = get_module(masked)
    return bass_utils.run_bass_kernel_spmd(
        nc, in_maps, core_ids=list(range(N_CORES)), **kwargs)


def kernel(x, mask, w_q, w_k, w_v, w_o, ln_gamma, ln_beta):
    in_maps, masked = prep_inputs(x, mask, w_q, w_k, w_v, w_o,
                                  ln_gamma, ln_beta)
    res = run(in_maps, masked)
    return assemble(res.results)
